# revision 40
# baseline (speedup 1.0000x reference)
"""Sliding-window causal self-attention (T=4096, D=1024, 16 heads,
window=1024) on 8 Trainium2 NeuronCores.

Sharding: tensor-parallel over heads - 2 heads per core. Each core computes
the qkv projection for its heads (sliced w_qkv rows), full attention for its
heads, and a partial output projection (sliced w_o columns, 0.5*v folded in).
The 8 partial [T, D] fp16 outputs are summed on the host (the all-reduce).

Design notes (v3):
- All matmuls run in bf16 (1 cycle/row at any output width on TRN2) with
  fp32 PSUM accumulation. Scores contract over K=64 per head directly.
- Phase A (projection + rmsnorm + rope) and phase B (attention + w_o) are
  software-pipelined, A two query-block pairs ahead of B, with emission
  interleaved at sub-unit granularity so all five engines see mixed work
  and the PE stays ramped.
- Engine budget (PSUM is reachable only from PE/Act/DVE/DMA-less paths):
  Pool gets all SBUF-only elementwise work (sq, rms-scale, masks, parts of
  rsqrt); Act gets exp + a share of PSUM drains; DVE gets the rest.
- rsqrt for rmsnorm is computed with a quake-III bit-trick seed + one
  Halley iteration on DVE/Pool - the Act engine then only ever runs Exp
  and Copy, so its activation table is never reloaded mid-kernel.
- rope runs BEFORE the 1/rms scaling (they commute; rotation also
  preserves the per-group sum of squares), taking it off the
  rsqrt -> scale -> transpose critical path. The pair-swap multiply is a
  single strided-output tensor op; cos/sin arrive sign-folded from the
  host.
- softmax normalization is fused into the PSUM drain of the p@v
  accumulator (reciprocal of the appended ones-column, then a
  tensor_scalar multiply on the way to SBUF).
- PSUM plan (8 banks): pq x2, sc x3, av/tpA/tpB x2, wo x1.
"""

from contextlib import ExitStack

import numpy as np
import ml_dtypes

import concourse.bass as bass
import concourse.mybir as mybir
import concourse.tile as tile
from concourse import bacc
from concourse.bass import ts
from concourse.bass_utils import run_bass_kernel_spmd
from concourse.masks import make_identity

F32 = mybir.dt.float32
BF16 = mybir.dt.bfloat16
FP16 = mybir.dt.float16

T = 4096
C = 1024
H = 16
DH = 64
N_CORES = 8
HPC = H // N_CORES  # heads per core = 2
NB = T // 128       # 32 token blocks
NP = NB // 2        # 16 query-block pairs
WINDOW = 1024
LOOKBACK = WINDOW // 128
SCALE = 0.12
EPS = 1e-6
ROT = DH // 2   # 32 rotary channels
PAIR = ROT // 2
LEAD = 2        # A-phase runs this many pairs ahead of B


def build_program(with_nw: bool):
    nc = bacc.Bacc("TRN2", target_bir_lowering=False, debug=False,
                   num_devices=N_CORES)

    xT = nc.dram_tensor("xT", [C, T], BF16, kind="ExternalInput").ap()
    wT = nc.dram_tensor("wT", [C, 6 * DH], BF16, kind="ExternalInput").ap()
    w_oT = nc.dram_tensor("w_oT", [2 * DH, C], BF16, kind="ExternalInput").ap()
    ab = nc.dram_tensor("ab", [T, 2, 4 * ROT], BF16, kind="ExternalInput").ap()
    # masks in scoresT layout [key j (partition), query i (free)]
    mf = nc.dram_tensor("mf", [128, 128], BF16, kind="ExternalInput").ap()
    ml = nc.dram_tensor("ml", [128, 128], BF16, kind="ExternalInput").ap()
    nw = None
    if with_nw:
        nw = nc.dram_tensor("nw", [128, 256], BF16, kind="ExternalInput").ap()
    outp = nc.dram_tensor("outp", [T, C], FP16, kind="ExternalOutput").ap()

    with tile.TileContext(nc) as tc:
        _body(tc, xT, wT, w_oT, ab, mf, ml, nw, outp)
    nc.compile()
    return nc


def _body(tc, xT, wT, w_oT, ab, mf, ml, nw, outp):
    nc = tc.nc
    with ExitStack() as octx:
        singles = octx.enter_context(tc.tile_pool(name="singles", bufs=1))

        # persistent SBUF residents
        # qkT: d-major q and k; partitions = 2 heads x 64 channels,
        # [:, 0, :] = q, [:, 1, :] = k
        qkT = singles.tile([128, 2, T], BF16)
        # v + ones column, token-major: [tok%128, block, head, 64+1]
        v_all = singles.tile([128, NB, HPC, DH + 1], BF16)
        wT_sb = singles.tile([128, 8, 6 * DH], BF16)
        w_oT_sb = singles.tile([128, C], BF16)
        mf_sb = singles.tile([128, 128], BF16)
        ml_sb = singles.tile([128, 128], BF16)
        identb = singles.tile([128, 128], BF16)
        eps_sb = singles.tile([128, 1], F32)
        # uint32 per-partition constants for the Newton-rsqrt bit trick
        c_xor = singles.tile([128, 1], mybir.dt.uint32)
        c_one = singles.tile([128, 1], mybir.dt.uint32)
        c_sub = singles.tile([128, 1], mybir.dt.uint32)
        nw_sb = None
        if nw is not None:
            nw_sb = singles.tile([128, 256], BF16)
            nc.sync.dma_start(nw_sb, nw)

        # wT first: the very first projection matmuls need it
        nc.sync.dma_start(wT_sb, wT.rearrange("(a p) n -> p a n", p=128))
        make_identity(nc, identb)
        nc.vector.memset(eps_sb, EPS)
        nc.vector.memset(v_all[:, :, :, DH:DH + 1], 1.0)
        nc.vector.memset(c_xor, 0xFFFFFFFF)
        nc.vector.memset(c_one, 1)
        nc.vector.memset(c_sub, 0x7FFFFFFF - 0x5F3759DF)

        xT_r = xT.rearrange("(a p) t -> p a t", p=128)

        with ExitStack() as bctx:
            ppq = bctx.enter_context(
                tc.tile_pool(name="ppq", bufs=1, space="PSUM"))
            psc = bctx.enter_context(
                tc.tile_pool(name="psc", bufs=2, space="PSUM"))
            pav = bctx.enter_context(
                tc.tile_pool(name="pav", bufs=2, space="PSUM"))
            pwo = bctx.enter_context(
                tc.tile_pool(name="pwo", bufs=1, space="PSUM"))
            awork = bctx.enter_context(tc.tile_pool(name="awork", bufs=3))
            bwork = bctx.enter_context(tc.tile_pool(name="bwork", bufs=2))
            bsmall = bctx.enter_context(tc.tile_pool(name="bsmall", bufs=4))

            a_tiles = {}      # pair -> (xt, ab_t)
            qkn_tiles = {}    # block -> qkn tile awaiting transpose
            attn_tiles = {}   # block -> attn tile
            aT_tiles = {}     # block -> aT tile
            wo_tiles = {}     # pair -> wo_sb tile
            deferred = []     # closures: wo chunks + out DMA

            def emit_a_dma(pp):
                xt = awork.tile([128, 8, 256], BF16, tag="xt", name="xt")
                nc.sync.dma_start(xt, xT_r[:, :, ts(pp, 256)])
                ab_t = awork.tile([128, 2, 2, 4 * ROT], BF16, tag="ab",
                                  name="ab_t")
                nc.sync.dma_start(
                    ab_t, ab[ts(pp, 256), :, :].rearrange(
                        "(b p) two r -> p b two r", p=128))
                a_tiles[pp] = (xt, ab_t)

            def emit_a_proj(i, ms_pair):
                pp, b = i // 2, i % 2
                xt, ab_t = a_tiles[pp]

                pq = ppq.tile([128, 6 * DH], F32, tag="pq", name="pq")
                for j in range(8):
                    nc.tensor.matmul(pq, lhsT=xt[:, j, ts(b, 128)],
                                     rhs=wT_sb[:, j, :],
                                     start=(j == 0), stop=(j == 7))

                # drain pq fast so its PSUM bank frees for the next pair:
                # qk part to SBUF (split Act/DVE), v part to v_all (DVE).
                # Scheduled at high priority so the ring-1 pq bank never
                # gates the next projection.
                qkv_sb = awork.tile([128, 256], F32, tag="qkv", name="qkv_sb")
                nc.scalar.copy(qkv_sb[:, 0:128], pq[:, 0:128])
                nc.vector.tensor_copy(qkv_sb[:, 128:256], pq[:, 128:256])
                nc.vector.tensor_copy(
                    v_all[:, i, :, 0:DH],
                    pq[:, 256:384].rearrange("p (h d) -> p h d", h=HPC))

                # rope BEFORE the rmsnorm scaling (they commute: the rotation
                # is per-token-orthogonal, so it neither changes the sum of
                # squares below nor depends on the 1/rms scale).
                # q' = q*A + swap(q)*B with B sign-folded
                # (A = cos interleaved, B[2f] = -sin, B[2f+1] = +sin)
                xt_, ab_t = a_tiles[pp]
                qk4 = qkv_sb.rearrange("p (g d) -> p g d", g=4)
                qr = qk4[:, :, 0:ROT].rearrange("p g (a two) -> p g a two",
                                                two=2)
                bv_ = ab_t[:, b, 1, :].rearrange("p (g a two) -> p g a two",
                                                 g=4, two=2)
                rot = awork.tile([128, 4, ROT], F32, tag="rot", name="rot")
                rot2 = rot.rearrange("p g (a two) -> p g a two", two=2)
                # one mul with pair-swapped OUTPUT AP: rot[swap(k)] = q[k]*C[k]
                rot_sw = rot2[:, :, :, ::-1]
                bv_sw = bv_[:, :, :, ::-1]
                nc.vector.tensor_mul(rot_sw, qr, bv_sw)
                f32v = qk4[:, :, 0:ROT]
                av_f = ab_t[:, b, 0, :].rearrange("p (g r) -> p g r", g=4)
                nc.vector.tensor_mul(f32v, f32v, av_f)
                nc.vector.tensor_add(f32v, f32v, rot)

                # sum of squares per q/k head group (64 ch each)
                sq = awork.tile([128, 4, DH], F32, tag="sq", name="sq")
                nc.gpsimd.tensor_mul(sq, qk4, qk4)
                nc.vector.reduce_sum(ms_pair[:, b, :], sq,
                                     axis=mybir.AxisListType.X)
                return qkv_sb

            def emit_rsqrt(ms_pair, rinv_pair):
                # rinv = rsqrt(ms/DH + eps), all on DVE (no Act table load):
                # quake-III seed (via xor/shift/sub, so the seed stays
                # positive) + two Newton iterations.
                msf = ms_pair.rearrange("p b f -> p (b f)")
                rvf = rinv_pair.rearrange("p b f -> p (b f)")
                var = awork.tile([128, 8], F32, tag="nvar", name="var")
                nc.vector.tensor_scalar(var, msf, 1.0 / DH, EPS,
                                        mybir.AluOpType.mult,
                                        mybir.AluOpType.add)
                y0 = awork.tile([128, 8], F32, tag="ny0", name="y0")
                y0u = y0.bitcast(mybir.dt.uint32)
                varu = var.bitcast(mybir.dt.uint32)
                cxb, _vu = bass.broadcast_tensor_aps(c_xor, varu)
                nc.vector.tensor_tensor(y0u, _vu, cxb,
                                        mybir.AluOpType.bitwise_xor)
                c1b, _yu = bass.broadcast_tensor_aps(c_one, y0u)
                nc.vector.tensor_tensor(y0u, _yu, c1b,
                                        mybir.AluOpType.logical_shift_right)
                csb, _yu2 = bass.broadcast_tensor_aps(c_sub, y0u)
                nc.vector.tensor_tensor(y0u, _yu2, csb,
                                        mybir.AluOpType.subtract)
                a1 = awork.tile([128, 8], F32, tag="na1", name="a1")
                for src, dst in ((y0, rvf), (rvf, rvf)):
                    nc.vector.tensor_mul(a1, src, src)
                    nc.vector.tensor_mul(a1, a1, var)
                    nc.vector.tensor_scalar(a1, a1, -0.5, 1.5,
                                            mybir.AluOpType.mult,
                                            mybir.AluOpType.add)
                    nc.vector.tensor_mul(dst, src, a1)

            def emit_a_finish(i, qkv_sb, rinv_pair):
                pp, b = i // 2, i % 2
                pq4 = qkv_sb.rearrange("p (g d) -> p g d", g=4)

                qkn = awork.tile([128, 256], BF16, tag="qkn", name="qkn",
                                 bufs=4)
                qkn4 = qkn.rearrange("p (g d) -> p g d", g=4)
                rv3 = rinv_pair[:, b, :].rearrange("p (f o) -> p f o", o=1)
                in0b, in1b = bass.broadcast_tensor_aps(pq4, rv3)
                nc.gpsimd.tensor_mul(qkn4, in0b, in1b)
                if nw_sb is not None:
                    nc.gpsimd.tensor_mul(qkn, qkn, nw_sb)
                qkn_tiles[i] = qkn

            def emit_a_transpose(i):
                # deferred so the rope DVE ops have cleared before PE needs
                # the transpose input
                qkn = qkn_tiles.pop(i)
                tpA = pav.tile([128, 2, 128], BF16, tag="av", name="tpA")
                nc.tensor.transpose(tpA[:, 0, :], qkn[:, 0:128], identb)
                nc.tensor.transpose(tpA[:, 1, :], qkn[:, 128:256], identb)
                nc.vector.tensor_copy(qkT[:, :, ts(i, 128)], tpA)

            pT_tiles = {}

            def emit_unit_sc(p, h):
                nkp = min(2 * p, LOOKBACK) + 2
                k0 = 2 * p + 1 - (nkp - 1)
                hp = 64 * h
                pT = bwork.tile([128, LOOKBACK + 2, 256], BF16, tag="pT",
                                name="pT")
                pT_tiles[(p, h)] = pT
                j = 0
                while j < nkp:
                    w = min(4, nkp - j)
                    sc = psc.tile([128, 4, 256], F32, tag="sc", name="sc")
                    for jj in range(w):
                        nc.tensor.matmul(
                            sc[:, jj, :],
                            lhsT=qkT[hp:hp + 64, 1, ts(k0 + j + jj, 128)],
                            rhs=qkT[hp:hp + 64, 0, ts(p, 256)],
                            start=True, stop=True)
                    nc.scalar.activation(pT[:, j:j + w, :], sc[:, 0:w, :],
                                         mybir.ActivationFunctionType.Exp,
                                         scale=SCALE)
                    # window-edge + causal masks (multiplicative, post-exp),
                    # emitted as soon as their chunk's exp is available
                    if j == 0 and p >= 4:
                        nc.gpsimd.tensor_mul(pT[:, 0, 0:128], pT[:, 0, 0:128],
                                             mf_sb)
                        nc.gpsimd.tensor_mul(pT[:, 1, 128:256],
                                             pT[:, 1, 128:256], mf_sb)
                    if j + w >= nkp:
                        nc.gpsimd.tensor_mul(pT[:, nkp - 2, 0:128],
                                             pT[:, nkp - 2, 0:128], ml_sb)
                        nc.gpsimd.tensor_mul(pT[:, nkp - 1, 128:256],
                                             pT[:, nkp - 1, 128:256], ml_sb)
                    j += w

            def emit_unit_av(p, h):
                nkp = min(2 * p, LOOKBACK) + 2
                k0 = 2 * p + 1 - (nkp - 1)
                hp = 64 * h
                pT = pT_tiles.pop((p, h))
                # p @ [v | 1] per 128-query half; normalize on PSUM drain.
                # Masked edge chunks accumulate last so the PE doesn't wait
                # on the mask ops.
                for q in (0, 1):
                    blk = 2 * p + q
                    lo = 1 if (q == 1 and p >= 4) else 0
                    hi = nkp - 2 if q == 0 else nkp - 1
                    av = pav.tile([128, DH + 1], F32, tag="av", name="av")
                    edge = {hi}
                    if p >= 4:
                        edge.add(q)
                    js = [x for x in range(lo, hi + 1) if x not in edge] + \
                         [x for x in range(lo, hi + 1) if x in edge]
                    for m, jx in enumerate(js):
                        nc.tensor.matmul(
                            av, lhsT=pT[:, jx, 128 * q:128 * q + 128],
                            rhs=v_all[:, k0 + jx, h, :],
                            start=(m == 0), stop=(m == len(js) - 1))
                    r = bsmall.tile([128, 1], F32, tag="r", name="r")
                    nc.vector.reciprocal_approx_fast(r, av[:, DH:DH + 1])
                    nc.vector.tensor_scalar_mul(
                        attn_tiles[blk][:, hp:hp + 64], av[:, 0:DH], r)

            def emit_tail_tp(blk):
                at = attn_tiles.pop(blk)
                tpB = pav.tile([128, 128], BF16, tag="av", name="tpB")
                nc.tensor.transpose(tpB, at, identb)
                aT = bsmall.tile([128, 128], BF16, tag="aT", name="aT")
                nc.vector.tensor_copy(aT, tpB)
                aT_tiles[blk] = aT

            def make_wo_chunk(p, blk, half, drain_eng, last):
                b = blk % 2

                def emit():
                    aT = aT_tiles[blk]
                    wo_sb = wo_tiles[p]
                    wo = pwo.tile([128, 512], F32, tag="wo", name="wo")
                    nc.tensor.matmul(wo, lhsT=aT,
                                     rhs=w_oT_sb[:, ts(half, 512)],
                                     start=True, stop=True)
                    if drain_eng == "v":
                        nc.vector.tensor_copy(
                            wo_sb[:, b, ts(half, 512)], wo)
                    else:
                        nc.scalar.copy(wo_sb[:, b, ts(half, 512)], wo)
                    if last:
                        aT_tiles.pop(blk, None)
                        nc.sync.dma_start(
                            outp[ts(p, 256), :].rearrange(
                                "(b pp) c -> pp b c", pp=128),
                            wo_sb)
                        wo_tiles.pop(p, None)
                return emit

            def flush_n(k):
                for _ in range(k):
                    if deferred:
                        deferred.pop(0)()

            def flush_all():
                while deferred:
                    deferred.pop(0)()

            def emit_a_pair_full(pp):
                ms_pair = awork.tile([128, 2, 4], F32, tag="msp", name="msp")
                rinv_pair = awork.tile([128, 2, 4], F32, tag="rvp",
                                       name="rvp")
                qkv_a = emit_a_proj(2 * pp, ms_pair)
                qkv_b = emit_a_proj(2 * pp + 1, ms_pair)
                emit_rsqrt(ms_pair, rinv_pair)
                emit_a_finish(2 * pp, qkv_a, rinv_pair)
                emit_a_finish(2 * pp + 1, qkv_b, rinv_pair)
                emit_a_transpose(2 * pp)
                emit_a_transpose(2 * pp + 1)

            # ---- software pipeline: A runs LEAD pairs ahead of B,
            # DMAs one more pair ahead of that ----
            for pp in range(min(LEAD + 1, NP)):
                emit_a_dma(pp)
                if pp == 0:
                    # lower-priority loads queued behind the first x tile
                    nc.sync.dma_start(w_oT_sb, w_oT)
                    nc.sync.dma_start(mf_sb, mf)
                    nc.sync.dma_start(ml_sb, ml)
            for pp in range(min(LEAD, NP)):
                emit_a_pair_full(pp)

            # wo-drain engine pattern balances DVE/Act
            drain_pat = ["v", "a"]
            tp_pend = []

            for p in range(NP):
                ap = p + LEAD
                st = None
                attn_tiles[2 * p] = bsmall.tile([128, 128], BF16, tag="attn",
                                                name="attn_a")
                attn_tiles[2 * p + 1] = bsmall.tile([128, 128], BF16,
                                                    tag="attn", name="attn_b")
                wo_tiles[p] = bsmall.tile([128, 2, C], FP16, tag="wo_sb",
                                          name="wo_sb", bufs=3)
                emit_unit_sc(p, 0)
                if ap < NP:
                    if ap + 1 < NP:
                        emit_a_dma(ap + 1)
                    ms_pair = awork.tile([128, 2, 4], F32, tag="msp",
                                         name="msp")
                    rinv_pair = awork.tile([128, 2, 4], F32, tag="rvp",
                                           name="rvp")
                    qkv_a = emit_a_proj(2 * ap, ms_pair)
                    st = True
                while tp_pend:
                    emit_a_transpose(tp_pend.pop(0))
                flush_n(1)
                emit_unit_av(p, 0)
                if st:
                    qkv_b = emit_a_proj(2 * ap + 1, ms_pair)
                    emit_rsqrt(ms_pair, rinv_pair)
                flush_n(1)
                emit_unit_sc(p, 1)
                flush_n(2)
                emit_unit_av(p, 1)
                if st:
                    emit_a_finish(2 * ap, qkv_a, rinv_pair)
                    emit_a_finish(2 * ap + 1, qkv_b, rinv_pair)
                    a_tiles.pop(ap, None)
                    tp_pend += [2 * ap, 2 * ap + 1]
                flush_n(2)
                deferred.append(lambda blk=2 * p: emit_tail_tp(blk))
                deferred.append(lambda blk=2 * p + 1: emit_tail_tp(blk))
                for idx, (blk, half) in enumerate(
                        [(2 * p, 0), (2 * p, 1), (2 * p + 1, 0),
                         (2 * p + 1, 1)]):
                    deferred.append(make_wo_chunk(
                        p, blk, half, drain_pat[idx % 2],
                        last=(idx == 3)))
            flush_all()


_PROGRAMS = {}


def _get_program(with_nw=False):
    if with_nw not in _PROGRAMS:
        _PROGRAMS[with_nw] = build_program(with_nw)
    return _PROGRAMS[with_nw]


def _bf16(a):
    return np.ascontiguousarray(np.asarray(a, np.float32)).astype(
        ml_dtypes.bfloat16)


def make_in_maps(x, w_qkv, w_o, cos, sin, pos):
    """Host-side sharding: build the per-core input dicts."""
    xTb = _bf16(np.asarray(x, np.float32).reshape(T, C).T)

    pos_i = np.asarray(pos).reshape(-1)
    cos_u = np.asarray(cos, np.float32)[pos_i]
    sin_u = np.asarray(sin, np.float32)[pos_i]
    a32 = np.empty((T, ROT), np.float32)
    a32[:, 0::2] = cos_u
    a32[:, 1::2] = cos_u
    b32 = np.empty((T, ROT), np.float32)
    b32[:, 0::2] = -sin_u
    b32[:, 1::2] = sin_u
    ab = np.stack([np.tile(a32, (1, 4)), np.tile(b32, (1, 4))], axis=1)
    ab_b = _bf16(ab)

    ones = np.ones((128, 128), np.float32)
    mf = _bf16(np.tril(ones, -1))  # allowed iff q_i < key_j
    ml = _bf16(np.triu(ones, 0))   # allowed iff q_i >= key_j

    w_qkv = np.asarray(w_qkv, np.float32)
    w_o = np.asarray(w_o, np.float32)
    in_maps = []
    for c in range(N_CORES):
        h0, h1 = HPC * c, HPC * c + 1
        rows = np.r_[h0 * DH:(h0 + 1) * DH, h1 * DH:(h1 + 1) * DH]
        w_shard = np.concatenate(
            [w_qkv[rows], w_qkv[C + rows], w_qkv[2 * C + rows]], axis=0)
        wT_c = _bf16(w_shard.T)
        w_oT_c = _bf16((0.5 * w_o[:, rows]).T)
        in_maps.append({
            "xT": xTb, "wT": wT_c, "w_oT": w_oT_c,
            "ab": ab_b, "mf": mf, "ml": ml,
        })
    return in_maps


def _norm_weight_tile(q_norm_w, k_norm_w):
    nwv = np.concatenate([np.tile(np.asarray(q_norm_w, np.float32), HPC),
                          np.tile(np.asarray(k_norm_w, np.float32), HPC)])
    return _bf16(np.broadcast_to(nwv, (128, 256)))


def kernel(x, tokens, pos, w_qkv, w_o, q_norm_w, k_norm_w, cos, sin,
           window_tokens, block_size):
    assert int(window_tokens) == WINDOW and int(block_size) == 128
    with_nw = not (np.all(np.asarray(q_norm_w) == 1.0)
                   and np.all(np.asarray(k_norm_w) == 1.0))
    nc = _get_program(with_nw)
    in_maps = make_in_maps(x, w_qkv, w_o, cos, sin, pos)
    if with_nw:
        nw_t = _norm_weight_tile(q_norm_w, k_norm_w)
        for m in in_maps:
            m["nw"] = nw_t

    res = run_bass_kernel_spmd(nc, in_maps, list(range(N_CORES)))
    out = np.zeros((T, C), np.float64)
    for c in range(N_CORES):
        out += np.asarray(res.results[c]["outp"]).astype(np.float64)
    return out.astype(np.float32).reshape(1, T, C)
